# revision 5
# baseline (speedup 1.0000x reference)
"""Causal self-attention (RoPE, 16 heads, S=4096, D=1024) on 8 Trainium2 cores.

Sharding: tensor-parallel over heads — core c computes heads 2c, 2c+1.
All matmuls in bf16 (fp32 PSUM accumulate). Per core:
  - q/k projections into [d, s] layout; v projected directly into [s, d]
    via per-128-column stationary x tiles (no PE transposes).
  - RoPE pair-swap via a PE permutation matmul (no strided SBUF-SBUF DMAs);
    rotation as q*cos + (P@q)*sin with the sign folded into the sin table.
  - Transposed-score attention: scores [k, q] per head; the two heads run
    concurrently on disjoint PE row-groups via tile_position. Softmax
    denominator folds into the PV matmul via a ones-column on V.
  - Row-parallel output projection producing a bf16 partial [S, D];
    host sums the 8 partials in fp32.
"""
import sys
import numpy as np

sys.path.insert(0, "/opt/trn_rl_repo")

import ml_dtypes

import concourse.bacc as bacc
import concourse.mybir as mybir
from concourse.tile import TileContext
from concourse.bass_utils import run_bass_kernel_spmd

FP = mybir.dt.float32
BF = mybir.dt.bfloat16
BF_NP = ml_dtypes.bfloat16

S = 4096          # sequence length
DM = 1024         # model dim
HD = 64           # head dim
NCORES = 8
ROPE_THETA = 10000.0
NQC = 8           # q chunks of 512
QW = 512
NKT = 32          # k tiles of 128
NDC = 8           # d-model chunks of 128

_CACHE = {}


def _build(repeat=1):
    nc = bacc.Bacc("TRN2", target_bir_lowering=False, debug=False,
                   num_devices=NCORES)

    xT = nc.dram_tensor("xT", [DM, S], BF, kind="ExternalInput")
    wq = nc.dram_tensor("wq", [DM, 128], BF, kind="ExternalInput")
    wk = nc.dram_tensor("wk", [DM, 128], BF, kind="ExternalInput")
    wv = nc.dram_tensor("wv", [DM, 128], BF, kind="ExternalInput")
    wo = nc.dram_tensor("wo", [128, DM], BF, kind="ExternalInput")
    cosm = nc.dram_tensor("cosm", [128, S], BF, kind="ExternalInput")
    sinm = nc.dram_tensor("sinm", [128, S], BF, kind="ExternalInput")
    perm = nc.dram_tensor("perm", [128, 128], BF, kind="ExternalInput")
    ident = nc.dram_tensor("ident", [128, 128], BF, kind="ExternalInput")
    OUT = nc.dram_tensor("OUT", [S, DM], BF, kind="ExternalOutput")

    with nc.allow_low_precision(reason="bf16 matmuls within rel-err budget"), \
         TileContext(nc) as tc:
        with tc.tile_pool(name="const", bufs=1) as cpool, \
             tc.tile_pool(name="big", bufs=1) as bpool, \
             tc.tile_pool(name="xt", bufs=3) as xpool, \
             tc.tile_pool(name="pt", bufs=3) as ptpool, \
             tc.tile_pool(name="work", bufs=2) as wpool, \
             tc.tile_pool(name="outp", bufs=2) as opool, \
             tc.tile_pool(name="ps", bufs=1, space="PSUM") as pspool:
          for _rep in range(repeat):
            wq_sb = cpool.tile([128, NDC, 128], BF, tag="wq")
            wk_sb = cpool.tile([128, NDC, 128], BF, tag="wk")
            wv_sb = cpool.tile([128, NDC, 128], BF, tag="wv")
            wo_sb = cpool.tile([128, DM], BF, tag="wo")
            cos_sb = cpool.tile([128, S], BF, tag="cos")
            sin_sb = cpool.tile([128, S], BF, tag="sin")
            pm_sb = cpool.tile([128, 128], BF, tag="perm")
            id_sb = cpool.tile([128, 128], BF, tag="ident")

            # weight shards arrive as [DM, 128] = W_shard.T; stage so chunk dc
            # holds contraction rows dc*128..dc*128+127 on the partition dim
            # projection weights first, then chunk 0's x tile, THEN the rope
            # tables / Wo (not needed until ~8us in) — so the first matmul
            # isn't queued behind 2.5MB of constants
            nc.sync.dma_start(
                wq_sb[:], wq[:].rearrange("(c p) e -> p c e", p=128))
            xt0 = xpool.tile([128, NDC, QW], BF, tag="xt")
            nc.sync.dma_start(
                xt0[:, 0:4, :],
                xT[0:512, 0:QW].rearrange("(c p) s -> p c s", p=128))
            nc.sync.dma_start(
                wk_sb[:], wk[:].rearrange("(c p) e -> p c e", p=128))
            nc.sync.dma_start(
                wv_sb[:], wv[:].rearrange("(c p) e -> p c e", p=128))
            nc.sync.dma_start(
                xt0[:, 4:8, :],
                xT[512:1024, 0:QW].rearrange("(c p) s -> p c s", p=128))
            nc.sync.dma_start(cos_sb[:], cosm[:])
            nc.sync.dma_start(sin_sb[:], sinm[:])
            nc.sync.dma_start(pm_sb[:], perm[:])
            nc.sync.dma_start(id_sb[:], ident[:])
            nc.sync.dma_start(wo_sb[:], wo[:])

            q_sb = bpool.tile([128, S], BF, tag="q")
            k_sb = bpool.tile([128, S], BF, tag="k")
            v_sb = bpool.tile([128, NKT, 130], BF, tag="v")
            o_sb = bpool.tile([128, S], BF, tag="o")

            # ones columns for the softmax-denominator rows of the PV matmuls
            nc.gpsimd.memset(v_sb[:, :, 64:65], 1.0)
            nc.gpsimd.memset(v_sb[:, :, 129:130], 1.0)

            # ---- projections: q/k as [d, s]; v directly as [s, d] per k-tile
            for sc in range(NQC):
                ssl = slice(sc * QW, (sc + 1) * QW)
                if sc == 0:
                    xt = xt0
                else:
                    xt = xpool.tile([128, NDC, QW], BF, tag="xt")
                    nc.sync.dma_start(
                        xt[:], xT[:, ssl].rearrange("(c p) s -> p c s", p=128))
                vt_tmp = wpool.tile([128, QW], BF, tag="vt")
                for w_sb, dst in ((wq_sb, q_sb[:, ssl]), (wk_sb, k_sb[:, ssl]),
                                  (wv_sb, vt_tmp[:])):
                    psp = pspool.tile([128, QW], FP, tag="mm", bufs=2)
                    for dc in range(NDC):
                        nc.tensor.matmul(psp[:], w_sb[:, dc, :], xt[:, dc, :],
                                         start=(dc == 0), stop=(dc == NDC - 1))
                    nc.vector.tensor_copy(dst, psp[:])
                # transpose vT [d, s] -> v [s, d] per k-tile on the PE
                for j in range(4):
                    kt = 4 * sc + j
                    pst = pspool.tile([128, 128], BF, tag="mm", bufs=2)
                    nc.tensor.transpose(pst[:], vt_tmp[:, j * 128:(j + 1) * 128],
                                        id_sb[:])
                    nc.vector.tensor_copy(v_sb[:, kt, 0:64], pst[:, 0:64])
                    nc.vector.tensor_copy(v_sb[:, kt, 65:129], pst[:, 64:128])
                # RoPE on q and k: t = t*cos + (P@t)*sin (sign baked into sin)
                for t_sb in (q_sb, k_sb):
                    psw = pspool.tile([128, QW], FP, tag="mm", bufs=2)
                    nc.tensor.matmul(psw[:], pm_sb[:], t_sb[:, ssl],
                                     start=True, stop=True)
                    t1 = wpool.tile([128, QW], BF, tag="t1")
                    t2 = wpool.tile([128, QW], BF, tag="t2")
                    nc.vector.tensor_tensor(t1[:], t_sb[:, ssl], cos_sb[:, ssl],
                                            mybir.AluOpType.mult)
                    nc.vector.tensor_tensor(t2[:], psw[:], sin_sb[:, ssl],
                                            mybir.AluOpType.mult)
                    nc.vector.tensor_tensor(t_sb[:, ssl], t1[:], t2[:],
                                            mybir.AluOpType.add)

            # ---- attention, transposed scores: sT[k, q], both heads packed
            # the normalize+Wo block of chunk qc-1 is deferred until after
            # the first 3 k-tile units of chunk qc, so the next chunk's
            # scores (which feed ACT) precede the 8 Wo matmuls in PE's
            # static order
            scale = 1.0 / np.sqrt(HD)
            pending_fin = None
            for qc in range(NQC):
                qsl = slice(qc * QW, (qc + 1) * QW)
                nkt = 4 * (qc + 1)
                pv0 = pspool.tile([65, QW], FP, tag="pv0", bufs=1)
                pv1 = pspool.tile([65, QW], FP, tag="pv1", bufs=1)
                # software-pipelined: PV(kt) is emitted one k-tile behind the
                # scores/exp so PE's static order is s0 s1 P0 s2 P1 ... —
                # scores(t+1) completes during exp(t) and ACT never waits on
                # the PV+scores chain between exps
                def emit_pv(kt, pt, coff):
                    # suffix-only accumulate is safe: diagonal tiles are
                    # never the start=True tile unless coff == 0
                    nc.tensor.matmul(pv0[:, coff:], v_sb[:, kt, 0:65],
                                     pt[:, 0, coff:],
                                     start=(kt == 0), stop=(kt == nkt - 1))
                    nc.tensor.matmul(pv1[:, coff:], v_sb[:, kt, 65:130],
                                     pt[:, 1, coff:],
                                     start=(kt == 0), stop=(kt == nkt - 1))

                prev = None
                for kt in range(nkt):
                    ksl = slice(kt * 128, (kt + 1) * 128)
                    coff = (kt - 4 * qc) * 128 if kt > 4 * qc else 0
                    qs0 = qc * QW + coff
                    ps_s = pspool.tile([128, 1024], FP, tag="s", bufs=2)
                    ps3 = ps_s[:].rearrange("p (h q) -> p h q", h=2)
                    nc.tensor.matmul(ps3[:, 0, coff:],
                                     k_sb[0:64, ksl],
                                     q_sb[0:64, qs0:(qc + 1) * QW],
                                     start=True, stop=True,
                                     tile_position=(0, 0))
                    nc.tensor.matmul(ps3[:, 1, coff:],
                                     k_sb[64:128, ksl],
                                     q_sb[64:128, qs0:(qc + 1) * QW],
                                     start=True, stop=True,
                                     tile_position=(64, 0))
                    pt = ptpool.tile([128, 2, 512], BF, tag="pt")
                    # diagonal tiles: columns < coff are fully above the
                    # causal boundary — skip their exp; the affine_select
                    # below writes fill=0 over that whole region anyway
                    nc.scalar.activation(pt[:, :, coff:], ps3[:, :, coff:],
                                         mybir.ActivationFunctionType.Exp,
                                         scale=scale)
                    if kt >= 4 * qc:  # diagonal tile: zero where k > q
                        # sliced to the suffix, the keep-condition is
                        # col' - chan >= 0 for every diagonal tile
                        for h in range(2):
                            nc.gpsimd.affine_select(
                                out=pt[:, h, coff:],
                                in_=pt[:, h, coff:],
                                compare_op=mybir.AluOpType.is_ge,
                                fill=0.0, base=0,
                                pattern=[[1, 512 - coff]],
                                channel_multiplier=-1)
                    if prev is not None:
                        emit_pv(*prev)
                    prev = (kt, pt, coff)
                    if kt == 2 and pending_fin is not None:
                        pending_fin()
                        pending_fin = None
                emit_pv(*prev)

                # stage pv to SBUF immediately so the PSUM banks free up for
                # the next q-chunk's PV accumulation (must stay here — the pv
                # slots are re-allocated by the next chunk's first PV)
                pvc = wpool.tile([65, 2, QW], FP, tag="pvc")
                nc.vector.tensor_copy(pvc[:, 0, :], pv0[:])
                nc.vector.tensor_copy(pvc[:, 1, :], pv1[:])

                def mk_finalize(qc, qsl, pvc):
                    def finalize():
                        r_sb = wpool.tile([1, 1024], FP, tag="r")
                        nc.vector.reciprocal(r_sb[0:1, 0:512],
                                             pvc[64:65, 0, :])
                        nc.vector.reciprocal(r_sb[0:1, 512:1024],
                                             pvc[64:65, 1, :])
                        bcs = []
                        for h in range(2):
                            bc = wpool.tile([64, QW], FP, tag="bc")
                            nc.gpsimd.partition_broadcast(
                                bc[:], r_sb[0:1, h * 512:(h + 1) * 512],
                                channels=64)
                            bcs.append(bc)
                        nc.vector.tensor_tensor(o_sb[0:64, qsl],
                                                pvc[0:64, 0, :], bcs[0][:],
                                                mybir.AluOpType.mult)
                        nc.vector.tensor_tensor(o_sb[64:128, qsl],
                                                pvc[0:64, 1, :], bcs[1][:],
                                                mybir.AluOpType.mult)
                        for j2 in range(4):
                            st = qc * 4 + j2
                            ot = opool.tile([128, DM], BF, tag="ot")
                            for eh in range(2):
                                pf = pspool.tile([128, QW], FP, tag="mm",
                                                 bufs=2, name="pf")
                                nc.tensor.matmul(
                                    pf[:], o_sb[:, st * 128:(st + 1) * 128],
                                    wo_sb[:, eh * 512:(eh + 1) * 512],
                                    start=True, stop=True)
                                nc.vector.tensor_copy(
                                    ot[:, eh * 512:(eh + 1) * 512], pf[:])
                            nc.sync.dma_start(
                                OUT[st * 128:(st + 1) * 128, :], ot[:])
                    return finalize

                pending_fin = mk_finalize(qc, qsl, pvc)
            pending_fin()

    nc.compile()
    return nc


def _host_prep(x, Wq, Wk, Wv, Wo):
    x = np.asarray(x, dtype=np.float32)
    Wq = np.asarray(Wq, dtype=np.float32)
    Wk = np.asarray(Wk, dtype=np.float32)
    Wv = np.asarray(Wv, dtype=np.float32)
    Wo = np.asarray(Wo, dtype=np.float32)

    xT = np.ascontiguousarray(x.reshape(S, DM).T).astype(BF_NP)

    # RoPE tables in the [d, s] layout (sign of the swap folded into sin)
    pos = np.arange(S, dtype=np.float32)
    inv_freq = (ROPE_THETA ** (-np.arange(0, HD, 2, dtype=np.float32) / HD))
    ang = pos[None, :] * inv_freq[:, None]          # [32, S]
    cos_p = np.cos(ang).astype(np.float32)
    sin_p = np.sin(ang).astype(np.float32)
    cosm = np.empty((128, S), np.float32)
    sinm = np.empty((128, S), np.float32)
    for h in range(2):
        b = h * HD
        cosm[b + 0:b + HD:2] = cos_p
        cosm[b + 1:b + HD:2] = cos_p
        sinm[b + 0:b + HD:2] = -sin_p
        sinm[b + 1:b + HD:2] = sin_p
    cosm = cosm.astype(BF_NP)
    sinm = sinm.astype(BF_NP)

    # pair-swap permutation: P[i, j] = 1 iff i == j ^ 1 (within each head)
    perm = np.zeros((128, 128), np.float32)
    idx = np.arange(128)
    perm[idx ^ 1, idx] = 1.0
    perm = perm.astype(BF_NP)
    identm = np.eye(128, dtype=np.float32).astype(BF_NP)

    in_maps = []
    for c in range(NCORES):
        rows = slice(128 * c, 128 * (c + 1))
        in_maps.append({
            "xT": xT,
            "wq": np.ascontiguousarray(Wq[rows, :].T).astype(BF_NP),
            "wk": np.ascontiguousarray(Wk[rows, :].T).astype(BF_NP),
            "wv": np.ascontiguousarray(Wv[rows, :].T).astype(BF_NP),
            "wo": np.ascontiguousarray(Wo[:, rows].T).astype(BF_NP),
            "cosm": cosm,
            "sinm": sinm,
            "perm": perm,
            "ident": identm,
        })
    return in_maps


def kernel(x, Wq, Wk, Wv, Wo, _trace=False, _trace_kwargs=None):
    if "nc" not in _CACHE:
        _CACHE["nc"] = _build()
    nc = _CACHE["nc"]
    in_maps = _host_prep(x, Wq, Wk, Wv, Wo)
    kw = {}
    if _trace:
        kw = dict(trace=True, **(_trace_kwargs or {}))
    res = run_bass_kernel_spmd(nc, in_maps, core_ids=list(range(NCORES)), **kw)
    out = np.zeros((S, DM), np.float32)
    for r in res.results:
        out += np.asarray(r["OUT"], dtype=np.float32)
    _CACHE["last_results"] = res
    return out.astype(np.float32).reshape(1, S, DM)


# revision 6
# speedup vs baseline: 4.6387x; 4.6387x over previous
"""Causal self-attention (RoPE, 16 heads, S=4096, D=1024) on 8 Trainium2 cores.

Sharding: tensor-parallel over heads — core c computes heads 2c, 2c+1.
All matmuls in bf16 (fp32 PSUM accumulate). Per core:
  - q/k projections into [d, s] layout; v projected directly into [s, d]
    via per-128-column stationary x tiles (no PE transposes).
  - RoPE pair-swap via a PE permutation matmul (no strided SBUF-SBUF DMAs);
    rotation as q*cos + (P@q)*sin with the sign folded into the sin table.
  - Transposed-score attention: scores [k, q] per head; the two heads run
    concurrently on disjoint PE row-groups via tile_position. Softmax
    denominator folds into the PV matmul via a ones-column on V.
  - Row-parallel output projection producing a bf16 partial [S, D];
    host sums the 8 partials in fp32.
"""
import sys
import numpy as np

sys.path.insert(0, "/opt/trn_rl_repo")

import ml_dtypes

import concourse.bacc as bacc
import concourse.mybir as mybir
from concourse.tile import TileContext
from concourse.bass_utils import run_bass_kernel_spmd

FP = mybir.dt.float32
BF = mybir.dt.bfloat16
BF_NP = ml_dtypes.bfloat16

S = 4096          # sequence length
DM = 1024         # model dim
HD = 64           # head dim
NCORES = 8
ROPE_THETA = 10000.0
NQC = 8           # q chunks of 512
QW = 512
NKT = 32          # k tiles of 128
NDC = 8           # d-model chunks of 128

_CACHE = {}


def _build(repeat=1):
    nc = bacc.Bacc("TRN2", target_bir_lowering=False, debug=False,
                   num_devices=NCORES)

    xT = nc.dram_tensor("xT", [DM, S], BF, kind="ExternalInput")
    wq = nc.dram_tensor("wq", [DM, 128], BF, kind="ExternalInput")
    wk = nc.dram_tensor("wk", [DM, 128], BF, kind="ExternalInput")
    wv = nc.dram_tensor("wv", [DM, 128], BF, kind="ExternalInput")
    wo = nc.dram_tensor("wo", [128, DM], BF, kind="ExternalInput")
    cosm = nc.dram_tensor("cosm", [128, S], BF, kind="ExternalInput")
    sinm = nc.dram_tensor("sinm", [128, S], BF, kind="ExternalInput")
    perm = nc.dram_tensor("perm", [128, 128], BF, kind="ExternalInput")
    ident = nc.dram_tensor("ident", [128, 128], BF, kind="ExternalInput")
    OUT = nc.dram_tensor("OUT", [S, DM], BF, kind="ExternalOutput")

    with nc.allow_low_precision(reason="bf16 matmuls within rel-err budget"), \
         TileContext(nc) as tc:
        with tc.tile_pool(name="const", bufs=1) as cpool, \
             tc.tile_pool(name="big", bufs=1) as bpool, \
             tc.tile_pool(name="xt", bufs=4) as xpool, \
             tc.tile_pool(name="pt", bufs=4) as ptpool, \
             tc.tile_pool(name="work", bufs=3) as wpool, \
             tc.tile_pool(name="outp", bufs=3) as opool, \
             tc.tile_pool(name="ps", bufs=1, space="PSUM") as pspool:
          for _rep in range(repeat):
            wq_sb = cpool.tile([128, NDC, 128], BF, tag="wq")
            wk_sb = cpool.tile([128, NDC, 128], BF, tag="wk")
            wv_sb = cpool.tile([128, NDC, 128], BF, tag="wv")
            wo_sb = cpool.tile([128, DM], BF, tag="wo")
            cos_sb = cpool.tile([128, S], BF, tag="cos")
            sin_sb = cpool.tile([128, S], BF, tag="sin")
            pm_sb = cpool.tile([128, 128], BF, tag="perm")
            id_sb = cpool.tile([128, 128], BF, tag="ident")

            # weight shards arrive as [DM, 128] = W_shard.T; stage so chunk dc
            # holds contraction rows dc*128..dc*128+127 on the partition dim
            # projection weights first, then chunk 0's x tile, THEN the rope
            # tables / Wo (not needed until ~8us in) — so the first matmul
            # isn't queued behind 2.5MB of constants
            nc.sync.dma_start(
                wq_sb[:], wq[:].rearrange("(c p) e -> p c e", p=128))
            xt0 = xpool.tile([128, NDC, QW], BF, tag="xt")
            nc.sync.dma_start(
                xt0[:, 0:4, :],
                xT[0:512, 0:QW].rearrange("(c p) s -> p c s", p=128))
            nc.sync.dma_start(
                wk_sb[:], wk[:].rearrange("(c p) e -> p c e", p=128))
            nc.sync.dma_start(
                wv_sb[:], wv[:].rearrange("(c p) e -> p c e", p=128))
            nc.sync.dma_start(
                xt0[:, 4:8, :],
                xT[512:1024, 0:QW].rearrange("(c p) s -> p c s", p=128))
            nc.sync.dma_start(cos_sb[:], cosm[:])
            nc.sync.dma_start(sin_sb[:], sinm[:])
            nc.sync.dma_start(pm_sb[:], perm[:])
            nc.sync.dma_start(id_sb[:], ident[:])
            nc.sync.dma_start(wo_sb[:], wo[:])

            q_sb = bpool.tile([128, S], BF, tag="q")
            k_sb = bpool.tile([128, S], BF, tag="k")
            v_sb = bpool.tile([128, NKT, 130], BF, tag="v")
            o_sb = bpool.tile([128, S], BF, tag="o")

            # ones columns for the softmax-denominator rows of the PV matmuls
            nc.gpsimd.memset(v_sb[:, :, 64:65], 1.0)
            nc.gpsimd.memset(v_sb[:, :, 129:130], 1.0)

            # ---- projections: q/k as [d, s]; v directly as [s, d] per k-tile
            for sc in range(NQC):
                ssl = slice(sc * QW, (sc + 1) * QW)
                if sc == 0:
                    xt = xt0
                else:
                    xt = xpool.tile([128, NDC, QW], BF, tag="xt")
                    nc.sync.dma_start(
                        xt[:], xT[:, ssl].rearrange("(c p) s -> p c s", p=128))
                vt_tmp = wpool.tile([128, QW], BF, tag="vt")
                for w_sb, dst in ((wq_sb, q_sb[:, ssl]), (wk_sb, k_sb[:, ssl]),
                                  (wv_sb, vt_tmp[:])):
                    psp = pspool.tile([128, QW], FP, tag="mm", bufs=2)
                    for dc in range(NDC):
                        nc.tensor.matmul(psp[:], w_sb[:, dc, :], xt[:, dc, :],
                                         start=(dc == 0), stop=(dc == NDC - 1))
                    nc.vector.tensor_copy(dst, psp[:])
                # transpose vT [d, s] -> v [s, d] per k-tile on the PE
                for j in range(4):
                    kt = 4 * sc + j
                    pst = pspool.tile([128, 128], BF, tag="mm", bufs=2)
                    nc.tensor.transpose(pst[:], vt_tmp[:, j * 128:(j + 1) * 128],
                                        id_sb[:])
                    nc.vector.tensor_copy(v_sb[:, kt, 0:64], pst[:, 0:64])
                    nc.vector.tensor_copy(v_sb[:, kt, 65:129], pst[:, 64:128])
                # RoPE on q and k: t = t*cos + (P@t)*sin (sign baked into sin)
                for t_sb in (q_sb, k_sb):
                    psw = pspool.tile([128, QW], FP, tag="mm", bufs=2)
                    nc.tensor.matmul(psw[:], pm_sb[:], t_sb[:, ssl],
                                     start=True, stop=True)
                    t1 = wpool.tile([128, QW], BF, tag="t1")
                    t2 = wpool.tile([128, QW], BF, tag="t2")
                    nc.vector.tensor_tensor(t1[:], t_sb[:, ssl], cos_sb[:, ssl],
                                            mybir.AluOpType.mult)
                    nc.vector.tensor_tensor(t2[:], psw[:], sin_sb[:, ssl],
                                            mybir.AluOpType.mult)
                    nc.vector.tensor_tensor(t_sb[:, ssl], t1[:], t2[:],
                                            mybir.AluOpType.add)

            # ---- attention, transposed scores: sT[k, q], both heads packed
            # the normalize+Wo block of chunk qc-1 is deferred until after
            # the first 3 k-tile units of chunk qc, so the next chunk's
            # scores (which feed ACT) precede the 8 Wo matmuls in PE's
            # static order
            scale = 1.0 / np.sqrt(HD)
            pending_fin = None
            for qc in range(NQC):
                qsl = slice(qc * QW, (qc + 1) * QW)
                nkt = 4 * (qc + 1)
                pv0 = pspool.tile([65, QW], FP, tag="pv0", bufs=1)
                pv1 = pspool.tile([65, QW], FP, tag="pv1", bufs=1)
                # software-pipelined: PV(kt) is emitted one k-tile behind the
                # scores/exp so PE's static order is s0 s1 P0 s2 P1 ... —
                # scores(t+1) completes during exp(t) and ACT never waits on
                # the PV+scores chain between exps
                def emit_pv(kt, pt, coff):
                    # suffix-only accumulate is safe: diagonal tiles are
                    # never the start=True tile unless coff == 0
                    nc.tensor.matmul(pv0[:, coff:], v_sb[:, kt, 0:65],
                                     pt[:, 0, coff:],
                                     start=(kt == 0), stop=(kt == nkt - 1))
                    nc.tensor.matmul(pv1[:, coff:], v_sb[:, kt, 65:130],
                                     pt[:, 1, coff:],
                                     start=(kt == 0), stop=(kt == nkt - 1))

                prev = None
                for kt in range(nkt):
                    ksl = slice(kt * 128, (kt + 1) * 128)
                    coff = (kt - 4 * qc) * 128 if kt > 4 * qc else 0
                    qs0 = qc * QW + coff
                    ps_s = pspool.tile([128, 1024], FP, tag="s", bufs=2)
                    ps3 = ps_s[:].rearrange("p (h q) -> p h q", h=2)
                    nc.tensor.matmul(ps3[:, 0, coff:],
                                     k_sb[0:64, ksl],
                                     q_sb[0:64, qs0:(qc + 1) * QW],
                                     start=True, stop=True,
                                     tile_position=(0, 0))
                    nc.tensor.matmul(ps3[:, 1, coff:],
                                     k_sb[64:128, ksl],
                                     q_sb[64:128, qs0:(qc + 1) * QW],
                                     start=True, stop=True,
                                     tile_position=(64, 0))
                    pt = ptpool.tile([128, 2, 512], BF, tag="pt")
                    # diagonal tiles: columns < coff are fully above the
                    # causal boundary — skip their exp; the affine_select
                    # below writes fill=0 over that whole region anyway
                    nc.scalar.activation(pt[:, :, coff:], ps3[:, :, coff:],
                                         mybir.ActivationFunctionType.Exp,
                                         scale=scale)
                    if kt >= 4 * qc:  # diagonal tile: zero where k > q
                        # sliced to the suffix, the keep-condition is
                        # col' - chan >= 0 for every diagonal tile
                        for h in range(2):
                            nc.gpsimd.affine_select(
                                out=pt[:, h, coff:],
                                in_=pt[:, h, coff:],
                                compare_op=mybir.AluOpType.is_ge,
                                fill=0.0, base=0,
                                pattern=[[1, 512 - coff]],
                                channel_multiplier=-1)
                    if prev is not None:
                        emit_pv(*prev)
                    prev = (kt, pt, coff)
                    if kt == 2 and pending_fin is not None:
                        pending_fin()
                        pending_fin = None
                emit_pv(*prev)

                # stage pv to SBUF immediately so the PSUM banks free up for
                # the next q-chunk's PV accumulation (must stay here — the pv
                # slots are re-allocated by the next chunk's first PV)
                pvc = wpool.tile([65, 2, QW], FP, tag="pvc")
                nc.vector.tensor_copy(pvc[:, 0, :], pv0[:])
                nc.vector.tensor_copy(pvc[:, 1, :], pv1[:])

                def mk_finalize(qc, qsl, pvc):
                    def finalize():
                        r_sb = wpool.tile([1, 1024], FP, tag="r")
                        nc.vector.reciprocal(r_sb[0:1, 0:512],
                                             pvc[64:65, 0, :])
                        nc.vector.reciprocal(r_sb[0:1, 512:1024],
                                             pvc[64:65, 1, :])
                        bcs = []
                        for h in range(2):
                            bc = wpool.tile([64, QW], FP, tag="bc")
                            nc.gpsimd.partition_broadcast(
                                bc[:], r_sb[0:1, h * 512:(h + 1) * 512],
                                channels=64)
                            bcs.append(bc)
                        nc.vector.tensor_tensor(o_sb[0:64, qsl],
                                                pvc[0:64, 0, :], bcs[0][:],
                                                mybir.AluOpType.mult)
                        nc.vector.tensor_tensor(o_sb[64:128, qsl],
                                                pvc[0:64, 1, :], bcs[1][:],
                                                mybir.AluOpType.mult)
                        for j2 in range(4):
                            st = qc * 4 + j2
                            ot = opool.tile([128, DM], BF, tag="ot")
                            for eh in range(2):
                                pf = pspool.tile([128, QW], FP, tag="mm",
                                                 bufs=2, name="pf")
                                nc.tensor.matmul(
                                    pf[:], o_sb[:, st * 128:(st + 1) * 128],
                                    wo_sb[:, eh * 512:(eh + 1) * 512],
                                    start=True, stop=True)
                                nc.vector.tensor_copy(
                                    ot[:, eh * 512:(eh + 1) * 512], pf[:])
                            nc.sync.dma_start(
                                OUT[st * 128:(st + 1) * 128, :], ot[:])
                    return finalize

                pending_fin = mk_finalize(qc, qsl, pvc)
            pending_fin()

    nc.compile()
    return nc


def _host_prep(x, Wq, Wk, Wv, Wo):
    x = np.asarray(x, dtype=np.float32)
    Wq = np.asarray(Wq, dtype=np.float32)
    Wk = np.asarray(Wk, dtype=np.float32)
    Wv = np.asarray(Wv, dtype=np.float32)
    Wo = np.asarray(Wo, dtype=np.float32)

    xT = np.ascontiguousarray(x.reshape(S, DM).T).astype(BF_NP)

    # RoPE tables in the [d, s] layout (sign of the swap folded into sin)
    pos = np.arange(S, dtype=np.float32)
    inv_freq = (ROPE_THETA ** (-np.arange(0, HD, 2, dtype=np.float32) / HD))
    ang = pos[None, :] * inv_freq[:, None]          # [32, S]
    cos_p = np.cos(ang).astype(np.float32)
    sin_p = np.sin(ang).astype(np.float32)
    cosm = np.empty((128, S), np.float32)
    sinm = np.empty((128, S), np.float32)
    for h in range(2):
        b = h * HD
        cosm[b + 0:b + HD:2] = cos_p
        cosm[b + 1:b + HD:2] = cos_p
        sinm[b + 0:b + HD:2] = -sin_p
        sinm[b + 1:b + HD:2] = sin_p
    cosm = cosm.astype(BF_NP)
    sinm = sinm.astype(BF_NP)

    # pair-swap permutation: P[i, j] = 1 iff i == j ^ 1 (within each head)
    perm = np.zeros((128, 128), np.float32)
    idx = np.arange(128)
    perm[idx ^ 1, idx] = 1.0
    perm = perm.astype(BF_NP)
    identm = np.eye(128, dtype=np.float32).astype(BF_NP)

    in_maps = []
    for c in range(NCORES):
        rows = slice(128 * c, 128 * (c + 1))
        in_maps.append({
            "xT": xT,
            "wq": np.ascontiguousarray(Wq[rows, :].T).astype(BF_NP),
            "wk": np.ascontiguousarray(Wk[rows, :].T).astype(BF_NP),
            "wv": np.ascontiguousarray(Wv[rows, :].T).astype(BF_NP),
            "wo": np.ascontiguousarray(Wo[:, rows].T).astype(BF_NP),
            "cosm": cosm,
            "sinm": sinm,
            "perm": perm,
            "ident": identm,
        })
    return in_maps


def kernel(x, Wq, Wk, Wv, Wo, _trace=False, _trace_kwargs=None):
    if "nc" not in _CACHE:
        _CACHE["nc"] = _build()
    nc = _CACHE["nc"]
    in_maps = _host_prep(x, Wq, Wk, Wv, Wo)
    kw = {}
    if _trace:
        kw = dict(trace=True, **(_trace_kwargs or {}))
    res = run_bass_kernel_spmd(nc, in_maps, core_ids=list(range(NCORES)), **kw)
    out = np.zeros((S, DM), np.float32)
    for r in res.results:
        out += np.asarray(r["OUT"], dtype=np.float32)
    _CACHE["last_results"] = res
    return out.astype(np.float32).reshape(1, S, DM)
